# revision 19
# baseline (speedup 1.0000x reference)
"""
nn_CrossProdMean — Trainium2 Bass kernel (8 NeuronCores, data-parallel over batch).

Math:
    a = x @ Wa + ba                     # [b, n, hid]
    b = x @ Wb + bb                     # [b, n, hid]
    out = g * a * mean(b, -1, keepdims)

Key algebraic identity (exact):
    mean(b, -1) = x @ mean(Wb, axis=1) + mean(bb)
so the entire second matmul collapses to a matvec. Folding g into Wa/ba:
    out[t, h] = (x @ Wg + bg)[t, h] * m[t],   Wg = Wa * g,  bg = ba * g,
    m[t] = x[t] @ wbm + bbm,  wbm = Wb.mean(1), bbm = bb.mean()

Sharding: batch dim (8) -> one batch element per core. Weights replicated.

Device kernel (per core, x_c [4096, 1024]):
  - x is pre-transposed on host to xT [1024, 4096] so the contraction dim
    (1024) lands on SBUF partitions; xT 128x128 tiles are the stationary
    matmul operand, Wg columns the moving operand -> out tiles [tok, hid]
    are stored contiguously.
  - Matmuls run as float32r (1 cycle/row on the PE for N>=256, vs 4 for
    plain fp32).
  - epilogue on DVE: (+bias) then (*m per-partition scalar).
"""

import numpy as np

import concourse.bass as bass
import concourse.mybir as mybir
import concourse.tile as tile
from concourse import bacc
from concourse.bass_utils import run_bass_kernel_spmd

F32 = mybir.dt.float32
F32R = mybir.dt.float32r
F16 = mybir.dt.float16

N_CORES = 8
TOK = 4096          # tokens per core (batch element)
DIM = 1024
HID = 1024
P = 128
K_TILES = DIM // P          # 8
N_FREE = 512                # moving free dim per matmul
N_TILES = HID // N_FREE     # 2
SLAB = 512                  # tokens per x DMA slab
N_SLABS = TOK // SLAB       # 8
J_PER_SLAB = SLAB // P      # 4
MPAD = 2                    # moving-dim padding for the m matvec

# stationary (x) and moving (w) matmul dtypes. fp16 stationary uses a
# separate overlapped LDWEIGHTS (fp32r self-loads serially) and halves
# the x DMA traffic; fp32r moving keeps 12-bit weight mantissas.
X_DT = F16
W_DT = F32R
SPLIT_FIRST = 2      # 0: wg first; 2: slab-0 j0 chunk first, then weights
LAST_PER_J = True    # fine-grained stores on the last slab (shorter tail)
NO_M = False         # ablation: skip m matmuls (m := bbm)
NO_EPI = False       # ablation: skip bias/scale epilogue (copy psum->out)


def _build_module(reps=1):
    nc = bacc.Bacc("TRN2", target_bir_lowering=False, debug=False)

    xt = nc.dram_tensor("xt", [DIM, TOK], X_DT, kind="ExternalInput")
    wg = nc.dram_tensor("wg", [DIM, HID], W_DT, kind="ExternalInput")
    bg = nc.dram_tensor("bg", [P, HID], F32, kind="ExternalInput")
    wbm = nc.dram_tensor("wbm", [DIM, MPAD], W_DT, kind="ExternalInput")
    bbm = nc.dram_tensor("bbm", [P, 1], F32, kind="ExternalInput")
    out = nc.dram_tensor("out", [TOK, HID], F32, kind="ExternalOutput")

    xt_r = xt[:].rearrange("(k p) t -> p k t", p=P)
    wg_r = wg[:].rearrange("(k p) h -> p k h", p=P)
    wbm_r = wbm[:].rearrange("(k p) o -> p k o", p=P)
    out_r = out[:].rearrange("(s j p) h -> s p j h", p=P, j=J_PER_SLAB)

    with tile.TileContext(nc) as tc:
        with (
            tc.tile_pool(name="const", bufs=1) as const_pool,
            tc.tile_pool(name="xtp", bufs=3) as x_pool,
            tc.tile_pool(name="outp", bufs=2) as out_pool,
            tc.tile_pool(name="tmpp", bufs=4) as tmp_pool,
            tc.tile_pool(name="mp", bufs=8) as m_pool,
            tc.tile_pool(name="psy", bufs=6, space="PSUM") as psum_y_pool,
            tc.tile_pool(name="psm", bufs=2, space="PSUM") as psum_m_pool,
        ):

            def body():
                bbm_sb = const_pool.tile([P, 1], F32, tag="bbm")
                nc.sync.dma_start(bbm_sb[:], bbm[:])
                wbm_sb = const_pool.tile([P, K_TILES, MPAD], W_DT, tag="wbm")
                nc.sync.dma_start(wbm_sb[:], wbm_r)
                wg_sb = const_pool.tile([P, K_TILES, HID], W_DT, tag="wg")
                bg_sb = const_pool.tile([P, HID], F32, tag="bg")
                x0_sb = None
                nc.sync.dma_start(bg_sb[:], bg[:])
                if SPLIT_FIRST == 0:
                    nc.sync.dma_start(
                        wg_sb[:, :, 0:N_FREE], wg_r[:, :, 0:N_FREE])
                    nc.sync.dma_start(
                        wg_sb[:, :, N_FREE:HID], wg_r[:, :, N_FREE:HID])
                else:
                    x0_sb = x_pool.tile([P, K_TILES, SLAB], X_DT, tag="xt")
                    if SPLIT_FIRST == 1:     # xt halves around wg halves
                        cut = 2 * P
                        nc.sync.dma_start(
                            x0_sb[:, :, 0:cut], xt_r[:, :, 0:cut])
                        nc.sync.dma_start(
                            wg_sb[:, :, 0:N_FREE], wg_r[:, :, 0:N_FREE])
                        nc.sync.dma_start(
                            wg_sb[:, :, N_FREE:HID], wg_r[:, :, N_FREE:HID])
                        nc.sync.dma_start(
                            x0_sb[:, :, cut:SLAB], xt_r[:, :, cut:SLAB])
                    elif SPLIT_FIRST == 2:   # j0 chunk first
                        cut = P
                        nc.sync.dma_start(
                            x0_sb[:, :, 0:cut], xt_r[:, :, 0:cut])
                        nc.sync.dma_start(
                            wg_sb[:, :, 0:N_FREE], wg_r[:, :, 0:N_FREE])
                        nc.sync.dma_start(
                            wg_sb[:, :, N_FREE:HID], wg_r[:, :, N_FREE:HID])
                        nc.sync.dma_start(
                            x0_sb[:, :, cut:SLAB], xt_r[:, :, cut:SLAB])
                    else:                    # 3: whole xt0 before wg
                        nc.sync.dma_start(x0_sb[:], xt_r[:, :, 0:SLAB])
                        nc.sync.dma_start(
                            wg_sb[:, :, 0:N_FREE], wg_r[:, :, 0:N_FREE])
                        nc.sync.dma_start(
                            wg_sb[:, :, N_FREE:HID], wg_r[:, :, N_FREE:HID])

                for s in range(N_SLABS):
                    if s == 0 and x0_sb is not None:
                        xt_sb = x0_sb
                    else:
                        xt_sb = x_pool.tile([P, K_TILES, SLAB], X_DT,
                                            tag="xt")
                        nc.sync.dma_start(
                            xt_sb[:], xt_r[:, :, s * SLAB:(s + 1) * SLAB]
                        )
                    last_per_j = LAST_PER_J and s == N_SLABS - 1
                    out_sb = out_pool.tile([P, J_PER_SLAB, HID], F32,
                                           tag="os")
                    for j in range(J_PER_SLAB):
                        lhsT = [
                            xt_sb[:, k, j * P:(j + 1) * P]
                            for k in range(K_TILES)
                        ]
                        psum_y0 = psum_y_pool.tile([P, N_FREE], F32, tag="py")
                        psum_y1 = psum_y_pool.tile([P, N_FREE], F32, tag="py")
                        psum_m = psum_m_pool.tile([P, MPAD], F32, tag="pm")
                        for k in range(K_TILES):
                            st = k == 0
                            sp = k == K_TILES - 1
                            nc.tensor.matmul(
                                psum_y0[:],
                                lhsT[k],
                                wg_sb[:, k, 0:N_FREE],
                                start=st, stop=sp,
                            )
                            nc.tensor.matmul(
                                psum_y1[:],
                                lhsT[k],
                                wg_sb[:, k, N_FREE:HID],
                                start=st, stop=sp,
                            )
                            if not NO_M:
                                nc.tensor.matmul(
                                    psum_m[:],
                                    lhsT[k],
                                    wbm_sb[:, k, :],
                                    start=st, stop=sp,
                                )
                        m_sb = m_pool.tile([P, 1], F32, tag="m")
                        if NO_M:
                            nc.vector.tensor_copy(m_sb[:], bbm_sb[:])
                        else:
                            # ACT engine (idle): m = psum_m + bbm
                            nc.scalar.add(m_sb[:], psum_m[:, 0:1],
                                          bbm_sb[:])

                        for n, psum_y in ((0, psum_y0), (1, psum_y1)):
                            hs0 = n * N_FREE
                            if NO_EPI:
                                nc.vector.tensor_copy(
                                    out_sb[:, j, hs0:hs0 + N_FREE], psum_y[:])
                                continue
                            tmp = tmp_pool.tile([P, N_FREE], F32, tag="t")
                            nc.vector.tensor_add(
                                tmp[:], psum_y[:], bg_sb[:, hs0:hs0 + N_FREE]
                            )
                            nc.vector.tensor_scalar_mul(
                                out_sb[:, j, hs0:hs0 + N_FREE], tmp[:],
                                m_sb[:]
                            )
                        if last_per_j:
                            nc.sync.dma_start(out_r[s][:, j], out_sb[:, j])
                    if not last_per_j:
                        nc.sync.dma_start(out_r[s], out_sb[:])

            if reps == 1:
                body()
            else:
                with tc.For_i(0, reps, 1):
                    body()

    nc.compile()
    return nc


_NC = None


def _get_module():
    global _NC
    if _NC is None:
        _NC = _build_module()
    return _NC


def _round_fp32r(a):
    """Round fp32 array to fp32r (e8m11: RNE to 11 mantissa bits, low 12
    bits zeroed) — the PE's fast single-pass fp32 matmul input format."""
    u = np.ascontiguousarray(a, dtype=np.float32).view(np.uint32)
    lsb = (u >> 12) & 1
    r = (u + 0x7FF + lsb) & np.uint32(0xFFFFF000)
    return r.view(np.float32)


def _prep_inputs(x, Wa, ba, Wb, bb, g):
    x = np.asarray(x, dtype=np.float32)
    Wa = np.asarray(Wa, dtype=np.float32)
    ba = np.asarray(ba, dtype=np.float32)
    Wb = np.asarray(Wb, dtype=np.float32)
    bb = np.asarray(bb, dtype=np.float32)
    g = np.asarray(g, dtype=np.float32)

    round_w = _round_fp32r if W_DT == F32R else (
        lambda a: np.ascontiguousarray(a, dtype=np.float32))
    if X_DT == F16:
        round_x = lambda a: np.ascontiguousarray(a, dtype=np.float16)
    elif X_DT == F32R:
        round_x = _round_fp32r
    else:
        round_x = lambda a: np.ascontiguousarray(a, dtype=np.float32)

    wg = round_w(Wa * g[None, :])
    bg_row = ba * g
    bg = np.ascontiguousarray(np.broadcast_to(bg_row[None, :], (P, HID)))
    wbm_vec = Wb.mean(axis=1, dtype=np.float64).astype(np.float32)
    wbm_pad = np.zeros((DIM, MPAD), dtype=np.float32)
    wbm_pad[:, 0] = wbm_vec
    wbm = round_w(wbm_pad)
    bbm_val = np.float32(bb.mean(dtype=np.float64))
    bbm = np.full((P, 1), bbm_val, dtype=np.float32)

    in_maps = []
    for c in range(N_CORES):
        xt_c = round_x(np.ascontiguousarray(x[c].T))
        in_maps.append({
            "xt": xt_c, "wg": wg, "bg": bg, "wbm": wbm, "bbm": bbm,
        })
    return in_maps


def kernel(x, Wa, ba, Wb, bb, g):
    nc = _get_module()
    in_maps = _prep_inputs(x, Wa, ba, Wb, bb, g)
    res = run_bass_kernel_spmd(nc, in_maps, list(range(N_CORES)))
    out = np.stack([res.results[c]["out"] for c in range(N_CORES)], axis=0)
    return out


# revision 27
# speedup vs baseline: 1.2957x; 1.2957x over previous
"""
nn_CrossProdMean — Trainium2 Bass kernel (8 NeuronCores, data-parallel over batch).

Math:
    a = x @ Wa + ba                     # [b, n, hid]
    b = x @ Wb + bb                     # [b, n, hid]
    out = g * a * mean(b, -1, keepdims)

Key algebraic identity (exact):
    mean(b, -1) = x @ mean(Wb, axis=1) + mean(bb)
so the entire second matmul collapses to a matvec. Folding g into Wa/ba:
    out[t, h] = (x @ Wg + bg)[t, h] * m[t],   Wg = Wa * g,  bg = ba * g,
    m[t] = x[t] @ wbm + bbm,  wbm = Wb.mean(1), bbm = bb.mean()

Sharding: batch dim (8) -> one batch element per core. Weights replicated.

Device kernel (per core, x_c [4096, 1024]):
  - x is pre-transposed on host to xT [1024, 4096] so the contraction dim
    (1024) lands on SBUF partitions; xT 128x128 tiles are the stationary
    matmul operand, Wg columns the moving operand -> out tiles [tok, hid]
    are stored contiguously.
  - Matmuls run in fp16 (e5m10) with fp32 PSUM accumulation: 1 PE
    cycle/row AND a separate LDWEIGHTS that overlaps the previous matmul
    (fp32/fp32r matmuls self-load their stationary serially, costing
    ~107ns extra per matmul = ~55us/core; measured, not modeled).
    fp16 also halves the x/W DMA traffic. absmax rel err ~3.8e-4.
  - epilogue on DVE: (+bias) then (*m per-partition scalar); m's +bbm on
    the idle ACT engine.
"""

import numpy as np

import concourse.bass as bass
import concourse.mybir as mybir
import concourse.tile as tile
from concourse import bacc
from concourse.bass_utils import run_bass_kernel_spmd

F32 = mybir.dt.float32
F32R = mybir.dt.float32r
F16 = mybir.dt.float16

N_CORES = 8
TOK = 4096          # tokens per core (batch element)
DIM = 1024
HID = 1024
P = 128
K_TILES = DIM // P          # 8
N_FREE = 512                # moving free dim per matmul
N_TILES = HID // N_FREE     # 2
PSY_BUFS = 6 if N_TILES == 2 else 3
SLAB = 512                  # tokens per x DMA slab
N_SLABS = TOK // SLAB       # 8
J_PER_SLAB = SLAB // P      # 4
MPAD = 2                    # moving-dim padding for the m matvec

# stationary (x) and moving (w) matmul dtypes. fp16 stationary uses a
# separate overlapped LDWEIGHTS (fp32r self-loads serially) and halves
# the x DMA traffic; fp32r moving keeps 12-bit weight mantissas.
X_DT = F16
W_DT = F16
SPLIT_FIRST = 2      # 0: wg first; 2: slab-0 j0 chunk first, then weights
LAST_PER_J = True    # fine-grained stores on the last slab (shorter tail)
NO_M = False         # ablation: skip m matmuls (m := bbm)
NO_EPI = False       # ablation: skip bias/scale epilogue (copy psum->out)
UNROLL_REPS = False  # timing: python-unroll reps instead of For_i


def _build_module(reps=1):
    nc = bacc.Bacc("TRN2", target_bir_lowering=False, debug=False)

    xt = nc.dram_tensor("xt", [DIM, TOK], X_DT, kind="ExternalInput")
    wg = nc.dram_tensor("wg", [DIM, HID], W_DT, kind="ExternalInput")
    bg = nc.dram_tensor("bg", [P, HID], F32, kind="ExternalInput")
    wbm = nc.dram_tensor("wbm", [DIM, MPAD], W_DT, kind="ExternalInput")
    bbm = nc.dram_tensor("bbm", [P, 1], F32, kind="ExternalInput")
    out = nc.dram_tensor("out", [TOK, HID], F32, kind="ExternalOutput")

    xt_r = xt[:].rearrange("(k p) t -> p k t", p=P)
    wg_r = wg[:].rearrange("(k p) h -> p k h", p=P)
    wbm_r = wbm[:].rearrange("(k p) o -> p k o", p=P)
    out_r = out[:].rearrange("(s j p) h -> s p j h", p=P, j=J_PER_SLAB)

    with tile.TileContext(nc) as tc:
        with (
            tc.tile_pool(name="const", bufs=1) as const_pool,
            tc.tile_pool(name="xtp", bufs=4) as x_pool,
            tc.tile_pool(name="outp", bufs=3) as out_pool,
            tc.tile_pool(name="tmpp", bufs=6) as tmp_pool,
            tc.tile_pool(name="mp", bufs=8) as m_pool,
            tc.tile_pool(name="psy", bufs=PSY_BUFS, space="PSUM") as psum_y_pool,
            tc.tile_pool(name="psm", bufs=2, space="PSUM") as psum_m_pool,
        ):

            def body():
                bbm_sb = const_pool.tile([P, 1], F32, tag="bbm")
                nc.sync.dma_start(bbm_sb[:], bbm[:])
                wbm_sb = const_pool.tile([P, K_TILES, MPAD], W_DT, tag="wbm")
                nc.sync.dma_start(wbm_sb[:], wbm_r)
                wg_sb = const_pool.tile([P, K_TILES, HID], W_DT, tag="wg")
                bg_sb = const_pool.tile([P, HID], F32, tag="bg")
                x0_sb = None
                nc.sync.dma_start(bg_sb[:], bg[:])
                HH = HID // 2
                if SPLIT_FIRST == 0:
                    nc.sync.dma_start(
                        wg_sb[:, :, 0:HH], wg_r[:, :, 0:HH])
                    nc.sync.dma_start(
                        wg_sb[:, :, HH:HID], wg_r[:, :, HH:HID])
                else:
                    x0_sb = x_pool.tile([P, K_TILES, SLAB], X_DT, tag="xt")
                    if SPLIT_FIRST == 1:     # xt halves around wg halves
                        cut = 2 * P
                        nc.sync.dma_start(
                            x0_sb[:, :, 0:cut], xt_r[:, :, 0:cut])
                        nc.sync.dma_start(
                            wg_sb[:, :, 0:HH], wg_r[:, :, 0:HH])
                        nc.sync.dma_start(
                            wg_sb[:, :, HH:HID], wg_r[:, :, HH:HID])
                        nc.sync.dma_start(
                            x0_sb[:, :, cut:SLAB], xt_r[:, :, cut:SLAB])
                    elif SPLIT_FIRST == 2:   # j0 chunk first
                        cut = P
                        nc.sync.dma_start(
                            x0_sb[:, :, 0:cut], xt_r[:, :, 0:cut])
                        nc.sync.dma_start(
                            wg_sb[:, :, 0:HH], wg_r[:, :, 0:HH])
                        nc.sync.dma_start(
                            wg_sb[:, :, HH:HID], wg_r[:, :, HH:HID])
                        nc.sync.dma_start(
                            x0_sb[:, :, cut:SLAB], xt_r[:, :, cut:SLAB])
                    else:                    # 3: whole xt0 before wg
                        nc.sync.dma_start(x0_sb[:], xt_r[:, :, 0:SLAB])
                        nc.sync.dma_start(
                            wg_sb[:, :, 0:HH], wg_r[:, :, 0:HH])
                        nc.sync.dma_start(
                            wg_sb[:, :, HH:HID], wg_r[:, :, HH:HID])

                for s in range(N_SLABS):
                    if s == 0 and x0_sb is not None:
                        xt_sb = x0_sb
                    else:
                        xt_sb = x_pool.tile([P, K_TILES, SLAB], X_DT,
                                            tag="xt")
                        nc.sync.dma_start(
                            xt_sb[:], xt_r[:, :, s * SLAB:(s + 1) * SLAB]
                        )
                    last_per_j = LAST_PER_J and s == N_SLABS - 1
                    out_sb = out_pool.tile([P, J_PER_SLAB, HID], F32,
                                           tag="os")
                    for j in range(J_PER_SLAB):
                        lhsT = [
                            xt_sb[:, k, j * P:(j + 1) * P]
                            for k in range(K_TILES)
                        ]
                        psum_ys = [
                            psum_y_pool.tile([P, N_FREE], F32, tag="py",
                                             name=f"py{n}")
                            for n in range(N_TILES)
                        ]
                        psum_m = psum_m_pool.tile([P, MPAD], F32, tag="pm")
                        for k in range(K_TILES):
                            st = k == 0
                            sp = k == K_TILES - 1
                            for n, psum_y in enumerate(psum_ys):
                                nc.tensor.matmul(
                                    psum_y[:],
                                    lhsT[k],
                                    wg_sb[:, k,
                                          n * N_FREE:(n + 1) * N_FREE],
                                    start=st, stop=sp,
                                )
                            if not NO_M:
                                nc.tensor.matmul(
                                    psum_m[:],
                                    lhsT[k],
                                    wbm_sb[:, k, :],
                                    start=st, stop=sp,
                                )
                        m_sb = m_pool.tile([P, 1], F32, tag="m")
                        if NO_M:
                            nc.vector.tensor_copy(m_sb[:], bbm_sb[:])
                        else:
                            # ACT engine (idle): m = psum_m + bbm
                            nc.scalar.add(m_sb[:], psum_m[:, 0:1],
                                          bbm_sb[:])

                        for n, psum_y in enumerate(psum_ys):
                            hs0 = n * N_FREE
                            if NO_EPI:
                                nc.vector.tensor_copy(
                                    out_sb[:, j, hs0:hs0 + N_FREE], psum_y[:])
                                continue
                            tmp = tmp_pool.tile([P, N_FREE], F32, tag="t")
                            nc.vector.tensor_add(
                                tmp[:], psum_y[:], bg_sb[:, hs0:hs0 + N_FREE]
                            )
                            nc.vector.tensor_scalar_mul(
                                out_sb[:, j, hs0:hs0 + N_FREE], tmp[:],
                                m_sb[:]
                            )
                        if last_per_j:
                            nc.sync.dma_start(out_r[s][:, j], out_sb[:, j])
                    if not last_per_j:
                        nc.sync.dma_start(out_r[s], out_sb[:])

            if reps == 1:
                body()
            elif UNROLL_REPS:
                for _ in range(reps):
                    body()
            else:
                with tc.For_i(0, reps, 1):
                    body()

    nc.compile()
    return nc


_NC = None


def _get_module():
    global _NC
    if _NC is None:
        _NC = _build_module()
    return _NC


def _round_fp32r(a):
    """Round fp32 array to fp32r (e8m11: RNE to 11 mantissa bits, low 12
    bits zeroed) — the PE's fast single-pass fp32 matmul input format."""
    u = np.ascontiguousarray(a, dtype=np.float32).view(np.uint32)
    lsb = (u >> 12) & 1
    r = (u + 0x7FF + lsb) & np.uint32(0xFFFFF000)
    return r.view(np.float32)


def _prep_inputs(x, Wa, ba, Wb, bb, g):
    x = np.asarray(x, dtype=np.float32)
    Wa = np.asarray(Wa, dtype=np.float32)
    ba = np.asarray(ba, dtype=np.float32)
    Wb = np.asarray(Wb, dtype=np.float32)
    bb = np.asarray(bb, dtype=np.float32)
    g = np.asarray(g, dtype=np.float32)

    def _caster(dt):
        if dt == F32R:
            return _round_fp32r
        if dt == F16:
            return lambda a: np.ascontiguousarray(a, dtype=np.float16)
        return lambda a: np.ascontiguousarray(a, dtype=np.float32)

    round_w = _caster(W_DT)
    round_x = _caster(X_DT)

    wg = round_w(Wa * g[None, :])
    bg_row = ba * g
    bg = np.ascontiguousarray(np.broadcast_to(bg_row[None, :], (P, HID)))
    wbm_vec = Wb.mean(axis=1, dtype=np.float64).astype(np.float32)
    wbm_pad = np.zeros((DIM, MPAD), dtype=np.float32)
    wbm_pad[:, 0] = wbm_vec
    wbm = round_w(wbm_pad)
    bbm_val = np.float32(bb.mean(dtype=np.float64))
    bbm = np.full((P, 1), bbm_val, dtype=np.float32)

    in_maps = []
    for c in range(N_CORES):
        xt_c = round_x(np.ascontiguousarray(x[c].T))
        in_maps.append({
            "xt": xt_c, "wg": wg, "bg": bg, "wbm": wbm, "bbm": bbm,
        })
    return in_maps


def kernel(x, Wa, ba, Wb, bb, g):
    nc = _get_module()
    in_maps = _prep_inputs(x, Wa, ba, Wb, bb, g)
    res = run_bass_kernel_spmd(nc, in_maps, list(range(N_CORES)))
    out = np.stack([res.results[c]["out"] for c in range(N_CORES)], axis=0)
    return out
